# revision 24
# baseline (speedup 1.0000x reference)
"""Trainium2 Bass kernel for nn_CrossMambaFusionBlock (B=4, H=W=64, C=96,
d_inner=192, d_state=4, dt_rank=6, K=4 directions, 2 modalities).

Sharding: 8 NeuronCores = 4 batch samples x 2 modalities; each core computes
the full block output for one (sample, modality). The cross-modal C
projection is duplicated locally (the other modality's u is recomputed), so
no collectives are required.

Per-core layout:
  - d_inner = 192 split into d0 = 0:128 and d1 = 128:192. All d1 work uses
    "pair" tiles [128, .] whose halves carry two n-states (or duplicated
    data) so 64-wide work still runs 128 lanes wide.
  - L = 4096 runs along the free dim; the selective scan is one
    tensor_tensor_scan per (direction, state) chained across 1024-chunks.
    Reversed directions use negative-step APs; transposed directions use
    col-major strided views of the natural-layout tensors.
  - B/C row-broadcasts across partitions are SBUF->SBUF replication DMAs
    (step-0 second dim) from rows pre-placed at aligned partitions 0/32/64/96.
"""

import sys
import types
from contextlib import ExitStack

import ml_dtypes
import numpy as np

BF = ml_dtypes.bfloat16

B, H, W, C = 4, 64, 64, 96
DIN = 192
N = 4
R = 6
K = 4
L = H * W
D0, D1 = 128, 64
NCORE = 8
LCH = 1024
NCH = L // LCH
MMCH = 512
LN_EPS = 1e-5
PADW = (H + 2) * (W + 2)


def _install_ntff_hook():
    if "antenv.axon_hooks" in sys.modules:
        return
    try:
        import antenv.axon_hooks  # noqa: F401
        return
    except ImportError:
        pass
    try:
        mod = types.ModuleType("antenv.axon_hooks")
        _h = [None]
        mod.set_axon_ntff_profile_hook = lambda h: _h.__setitem__(0, h)
        mod.get_axon_ntff_profile_hook = lambda: _h[0]
        sys.modules["antenv.axon_hooks"] = mod
        import antenv

        antenv.axon_hooks = mod
        from trn_agent_boot.trn_boot import _ntff_profile_via_ctypes

        mod.set_axon_ntff_profile_hook(
            _ntff_profile_via_ctypes("/opt/axon/libaxon_pjrt.so")
        )
    except Exception:
        pass


_install_ntff_hook()

import concourse.hw_specs as _hw_specs  # noqa: E402

_orig_get_act_tables = _hw_specs.get_activation_tables


def _steered_act_tables(module_arch):
    """Compile-time steering only: report Exp/Ln as available solely in the
    combined natural_log_exp set so the table-load pass doesn't thrash
    between the exp-only and ln-only sets. Set ids/ordering unchanged."""
    tabs = _orig_get_act_tables(module_arch)
    import concourse.mybir as _mb

    combined = "natural_log_exp_and_others"
    if combined in tabs:
        for name, fns in tabs.items():
            if name != combined:
                fns.discard(_mb.ActivationFunctionType.Exp)
                fns.discard(_mb.ActivationFunctionType.Ln)
    return tabs


_hw_specs.get_activation_tables = _steered_act_tables

import concourse.bacc as bacc  # noqa: E402
import concourse.bass as bass  # noqa: E402
import concourse.mybir as mybir  # noqa: E402
import concourse.tile as tile  # noqa: E402
from concourse.bass_utils import run_bass_kernel_spmd  # noqa: E402

F32 = mybir.dt.float32
BF16 = mybir.dt.bfloat16
MUL = mybir.AluOpType.mult
ADD = mybir.AluOpType.add
SUB = mybir.AluOpType.subtract
AF = mybir.ActivationFunctionType

# engine assignment knobs (tuned against HW traces)
CFG = {
    "b_eng": "vector",       # b = dtu * B_bc
    "hc_eng": "vector",      # hc = h * C_bc
    "copy_eng": "scalar",    # PSUM -> SBUF compact copies
    "use_silu": True,
}


def _eng(nc, name):
    return getattr(nc, CFG[name.replace("%", "")]) if False else getattr(
        nc, CFG[name])


def _rep_ap(row_ap, n):
    """[1, F] aligned SBUF row -> [n, F] DMA-replication source AP."""
    ap = list(row_ap.ap)
    return bass.AP(tensor=row_ap.tensor, offset=row_ap.offset,
                   ap=[ap[0], [0, n]] + ap[1:])


def _v3(ap2d):
    """[p, LCH] flat -> [p, LCH//H, H] view (for col-major-matched ops)."""
    return ap2d.rearrange("p (a b) -> p a b", b=H)


def build_nc():
    nc = bacc.Bacc("TRN2", target_bir_lowering=False, debug=False,
                   num_devices=NCORE)

    def din(name, shape, dt=BF16):
        return nc.dram_tensor(name, shape, dt, kind="ExternalInput").ap()

    xpad_o = din("xpad_o", [C, PADW])
    xpad_t = din("xpad_t", [C, PADW])
    xnat_o = din("xnat_o", [L, C], F32)
    wf_o = din("wf_o", [C, 9 * 256])     # per tap: cols 0:128 d0; 128:256 d1 dup
    wf_t = din("wf_t", [C, 9 * 256])
    cb_o = din("cb_o", [1, 256])
    cb_t = din("cb_t", [1, 256])
    xpw_dt0 = din("xpw_dt0", [D0, K * R])
    xpw_dt1 = din("xpw_dt1", [D1, K * R])
    xpw_B0 = din("xpw_B0", [D0, K * 128])
    xpw_B1 = din("xpw_B1", [D1, K * 128])
    xpw_C0 = din("xpw_C0", [D0, K * 128])
    xpw_C1 = din("xpw_C1", [D1, K * 128])
    dtw_d0 = din("dtw_d0", [R, K * D0])
    dtw_d1p = din("dtw_d1p", [R, K * 128])
    dtb_d0 = din("dtb_d0", [D0, K], F32)
    dtb_d1p = din("dtb_d1p", [128, K], F32)
    dsum_d0 = din("dsum_d0", [D0, 1], F32)
    dsum_d1 = din("dsum_d1", [D1, 1], F32)
    ln_g0 = din("ln_g0", [D0, 1], F32)
    ln_g1 = din("ln_g1", [D1, 1], F32)
    ln_b0 = din("ln_b0", [D0, 1], F32)
    ln_b1 = din("ln_b1", [D1, 1], F32)
    woutT0 = din("woutT0", [D0, C])
    woutT1 = din("woutT1", [D1, C])
    out_o = nc.dram_tensor("out_o", [L, C], F32, kind="ExternalOutput").ap()
    bc_stage = nc.dram_tensor("bc_stage", [K, NCH, 8, LCH], BF16,
                              kind="Internal").ap()

    with tile.TileContext(nc, num_cores=NCORE, pool_alloc_mode="queue") as tc, \
            ExitStack() as ctx:
        cpool = ctx.enter_context(tc.tile_pool(name="consts", bufs=1))

        def ctile(name, src, shape, dt=BF16):
            t = cpool.tile(shape, dt, name=name)
            nc.sync.dma_start(t[:], src)
            return t

        w_xpw_dt0 = ctile("w_xpw_dt0", xpw_dt0[:], [D0, K * R])
        w_xpw_dt1 = ctile("w_xpw_dt1", xpw_dt1[:], [D1, K * R])
        w_xpw_B0 = ctile("w_xpw_B0", xpw_B0[:], [D0, K * 128])
        w_xpw_B1 = ctile("w_xpw_B1", xpw_B1[:], [D1, K * 128])
        w_xpw_C0 = ctile("w_xpw_C0", xpw_C0[:], [D0, K * 128])
        w_xpw_C1 = ctile("w_xpw_C1", xpw_C1[:], [D1, K * 128])
        w_dtw_d0 = ctile("w_dtw_d0", dtw_d0[:], [R, K * D0])
        w_dtw_d1p = ctile("w_dtw_d1p", dtw_d1p[:], [R, K * 128])
        w_dtb_d0 = ctile("w_dtb_d0", dtb_d0[:], [D0, K], F32)
        w_dtb_d1p = ctile("w_dtb_d1p", dtb_d1p[:], [128, K], F32)
        w_dsum0 = ctile("w_dsum0", dsum_d0[:], [D0, 1], F32)
        w_dsum1 = ctile("w_dsum1", dsum_d1[:], [D1, 1], F32)
        w_lng0 = ctile("w_lng0", ln_g0[:], [D0, 1], F32)
        w_lng1 = ctile("w_lng1", ln_g1[:], [D1, 1], F32)
        w_lnb0 = ctile("w_lnb0", ln_b0[:], [D0, 1], F32)
        w_lnb1 = ctile("w_lnb1", ln_b1[:], [D1, 1], F32)
        w_woutT0 = ctile("w_woutT0", woutT0[:], [D0, C])
        w_woutT1 = ctile("w_woutT1", woutT1[:], [D1, C])
        ones_row = cpool.tile([1, MMCH], BF16, name="ones_row")
        nc.vector.memset(ones_row[:], 1.0)
        mean_l0 = cpool.tile([D0, 128], BF16, name="mean_l0")
        nc.vector.memset(mean_l0[:], 1.0 / DIN)
        mean_l1 = cpool.tile([D1, 128], BF16, name="mean_l1")
        nc.vector.memset(mean_l1[:], 1.0 / DIN)
        eps_col = cpool.tile([128, 1], F32, name="eps_col")
        nc.vector.memset(eps_col[:], LN_EPS)

        big = ctx.enter_context(tc.tile_pool(name="big", bufs=1))
        u_o_d0 = big.tile([D0, L], BF16, name="u_o_d0")
        u_o_d1p = big.tile([128, L], BF16, name="u_o_d1p")
        u_t_d0 = big.tile([D0, L], BF16, name="u_t_d0")
        u_t_d1p = big.tile([128, L], BF16, name="u_t_d1p")
        y02_d0 = big.tile([D0, L], BF16, name="y02_d0")
        y13_d0 = big.tile([D0, L], BF16, name="y13_d0")
        y02_d1p = big.tile([128, L], BF16, name="y02_d1p")
        y13_d1p = big.tile([128, L], BF16, name="y13_d1p")

        # ================= stage A: in_proj (x) conv + silu =================
        with tc.tile_pool(name="stAw", bufs=1) as wpool, \
             tc.tile_pool(name="stA", bufs=2) as apool, \
             tc.tile_pool(name="stAps", bufs=2, space="PSUM") as apsum:

            def wtile(name, src_ap, shape):
                t = wpool.tile(shape, BF16, name=name)
                nc.sync.dma_start(t[:], src_ap)
                return t

            w_xpad_o = wtile("w_xpad_o", xpad_o[:], [C, PADW])
            w_xpad_t = wtile("w_xpad_t", xpad_t[:], [C, PADW])
            w_wf_o = wtile("w_wf_o", wf_o[:], [C, 9 * 256])
            w_wf_t = wtile("w_wf_t", wf_t[:], [C, 9 * 256])
            w_cb_o = wtile("w_cb_o", cb_o[:], [1, 256])
            w_cb_t = wtile("w_cb_t", cb_t[:], [1, 256])

            def stage_a(w_xpad, w_wf, w_cb, u_d0, u_d1p, colmajor=False):
                if colmajor:
                    # free dims (col, row): iterates w-major
                    xv = w_xpad[:].rearrange("c (r q) -> c q r", q=W + 2)
                else:
                    xv = w_xpad[:].rearrange("c (r q) -> c r q", q=W + 2)
                nrow = MMCH // W  # h-rows (or w-cols) per matmul chunk
                for ci in range(L // MMCH):
                    h0 = ci * nrow
                    dsts = ((0, u_d0), (128, u_d1p)) if u_d1p is not None \
                        else ((0, u_d0),)
                    for dof, u_dst in dsts:
                        ps = apsum.tile([128, MMCH], F32, name="ps_a",
                                        tag="ps_a")
                        nc.tensor.matmul(ps[:], w_cb[:, dof:dof + 128],
                                         ones_row[:], start=True, stop=False)
                        for tap in range(9):
                            dy, dx = tap // 3, tap % 3
                            if colmajor:
                                rhs = xv[:, h0 + dx:h0 + dx + nrow,
                                         dy:dy + H]
                            else:
                                rhs = xv[:, h0 + dy:h0 + dy + nrow, dx:dx + W]
                            wl = w_wf[:, tap * 256 + dof:tap * 256 + dof + 128]
                            nc.tensor.matmul(ps[:], wl, rhs,
                                             start=False, stop=(tap == 8))
                        sl = slice(ci * MMCH, (ci + 1) * MMCH)
                        if CFG["use_silu"]:
                            nc.scalar.activation(u_dst[:, sl], ps[:], AF.Silu,
                                                 bias=0.0, scale=1.0)
                        else:
                            sg = apool.tile([128, MMCH], BF16, name="sg",
                                            tag="sg")
                            nc.scalar.activation(sg[:], ps[:], AF.Sigmoid,
                                                 bias=0.0, scale=1.0)
                            nc.vector.tensor_tensor(u_dst[:, sl], sg[:],
                                                    ps[:], MUL)

            stage_a(w_xpad_o, w_wf_o, w_cb_o, u_o_d0, u_o_d1p)
            stage_a(w_xpad_t, w_wf_t, w_cb_t, u_t_d0, u_t_d1p)

        for t in (y02_d0, y13_d0, y02_d1p, y13_d1p):
            nc.vector.memset(t[:], 0.0)

        # ================= stage B: 4-direction selective scans =============
        with tc.tile_pool(name="dtp", bufs=2) as dtp, \
             tc.tile_pool(name="cmp", bufs=2) as cmp_p, \
             tc.tile_pool(name="bcp", bufs=2) as bcp, \
             tc.tile_pool(name="abp", bufs=1) as abp, \
             tc.tile_pool(name="hp", bufs=2) as hp, \
             tc.tile_pool(name="rop", bufs=1) as rop, \
             tc.tile_pool(name="bps", bufs=1, space="PSUM") as bps:

            def u_view(u_tile, k, c, part=None):
                """Chunk c (scan order) of u for direction k. 2D for k=0,2;
                3D [p, LCH//H, H] col-major for k=1,3."""
                tl = u_tile[:part, :] if part else u_tile[:]
                if k in (0, 2):
                    lc = c if k == 0 else NCH - 1 - c
                    return tl[:, lc * LCH:(lc + 1) * LCH]
                wv = tl.rearrange("p (h w) -> p w h", w=W)
                wc = c if k == 1 else NCH - 1 - c
                nwc = LCH // H
                return wv[:, wc * nwc:(wc + 1) * nwc, :]

            b_eng = getattr(nc, CFG["b_eng"])
            hc_eng = getattr(nc, CFG["hc_eng"])

            def cp_copy(dst, srcap):
                if CFG["copy_eng"] == "scalar":
                    nc.scalar.copy(dst, srcap)
                else:
                    nc.vector.tensor_copy(dst, srcap)
            carries = {}

            for k in range(K):
                rev = k >= 2
                for c in range(NCH):
                    # ---- projections + dt_proj (PE), psum per MMCH ----
                    dt6 = cmp_p.tile([R, LCH], BF16, name="dt6", tag="dt6")
                    B_sb = cmp_p.tile([128, LCH], BF16, name="B_sb",
                                      tag="B_sb")
                    C_sb = cmp_p.tile([128, LCH], BF16, name="C_sb",
                                      tag="C_sb")
                    e1_0 = dtp.tile([D0, LCH], BF16, name="e1_0", tag="e1_0")
                    e1_1 = dtp.tile([128, LCH], BF16, name="e1_1", tag="e1_1")
                    for mi in range(LCH // MMCH):
                        ms = slice(mi * MMCH, (mi + 1) * MMCH)
                        if k in (0, 2):
                            ro0 = u_view(u_o_d0, k, c)[:, ms]
                            ro1 = u_view(u_o_d1p, k, c, part=D1)[:, ms]
                            rt0 = u_view(u_t_d0, k, c)[:, ms]
                            rt1 = u_view(u_t_d1p, k, c, part=D1)[:, ms]
                        else:
                            nw = MMCH // H
                            s3 = slice(mi * nw, (mi + 1) * nw)
                            ro0 = u_view(u_o_d0, k, c)[:, s3, :]
                            ro1 = u_view(u_o_d1p, k, c, part=D1)[:, s3, :]
                            rt0 = u_view(u_t_d0, k, c)[:, s3, :]
                            rt1 = u_view(u_t_d1p, k, c, part=D1)[:, s3, :]
                        ps_dt6 = bps.tile([R, MMCH], F32, name="ps_dt6",
                                          tag="ps_small", bufs=1)
                        nc.tensor.matmul(
                            ps_dt6[:], w_xpw_dt0[:, k * R:(k + 1) * R],
                            ro0, start=True, stop=False)
                        nc.tensor.matmul(
                            ps_dt6[:], w_xpw_dt1[:, k * R:(k + 1) * R],
                            ro1, start=False, stop=True)
                        cp_copy(dt6[:, ms], ps_dt6[:])
                        ps_B = bps.tile([128, MMCH], F32, name="ps_B",
                                        tag="ps_B", bufs=2)
                        nc.tensor.matmul(
                            ps_B[:], w_xpw_B0[:, k * 128:(k + 1) * 128],
                            ro0, start=True, stop=False)
                        nc.tensor.matmul(
                            ps_B[:], w_xpw_B1[:, k * 128:(k + 1) * 128],
                            ro1, start=False, stop=True)
                        cp_copy(B_sb[:, ms], ps_B[:])
                        ps_C = bps.tile([128, MMCH], F32, name="ps_C",
                                        tag="ps_C", bufs=2)
                        nc.tensor.matmul(
                            ps_C[:], w_xpw_C0[:, k * 128:(k + 1) * 128],
                            rt0, start=True, stop=False)
                        nc.tensor.matmul(
                            ps_C[:], w_xpw_C1[:, k * 128:(k + 1) * 128],
                            rt1, start=False, stop=True)
                        cp_copy(C_sb[:, ms], ps_C[:])
                        # dt_proj for this sub-chunk
                        ps_dt0 = bps.tile([D0, MMCH], F32, name="ps_dt0",
                                          tag="ps_dt0", bufs=2)
                        nc.tensor.matmul(ps_dt0[:],
                                         w_dtw_d0[:, k * D0:(k + 1) * D0],
                                         dt6[:, ms], start=True, stop=True)
                        nc.scalar.activation(e1_0[:, ms], ps_dt0[:], AF.Exp,
                                             bias=w_dtb_d0[:, k:k + 1],
                                             scale=1.0)
                        ps_dt1 = bps.tile([128, MMCH], F32, name="ps_dt1",
                                          tag="ps_dt1", bufs=1)
                        nc.tensor.matmul(ps_dt1[:],
                                         w_dtw_d1p[:, k * 128:(k + 1) * 128],
                                         dt6[:, ms], start=True, stop=True)
                        nc.scalar.activation(e1_1[:, ms], ps_dt1[:], AF.Exp,
                                             bias=w_dtb_d1p[:, k:k + 1],
                                             scale=1.0)
                    dt_0 = dtp.tile([D0, LCH], BF16, name="dt_0", tag="dt_0")
                    nc.scalar.activation(dt_0[:], e1_0[:], AF.Ln, bias=1.0)
                    dt_1 = dtp.tile([128, LCH], BF16, name="dt_1", tag="dt_1")
                    nc.scalar.activation(dt_1[:], e1_1[:], AF.Ln, bias=1.0)

                    # ---- dtu ----
                    dtu_0 = dtp.tile([D0, LCH], BF16, name="dtu_0",
                                     tag="dtu_0")
                    dtu_1 = dtp.tile([128, LCH], BF16, name="dtu_1",
                                     tag="dtu_1")
                    uvo0 = u_view(u_o_d0, k, c)
                    uvo1 = u_view(u_o_d1p, k, c)
                    if k in (0, 2):
                        nc.vector.tensor_tensor(dtu_0[:], dt_0[:], uvo0, MUL)
                        nc.vector.tensor_tensor(dtu_1[:], dt_1[:], uvo1, MUL)
                    else:
                        nc.vector.tensor_tensor(_v3(dtu_0[:]), _v3(dt_0[:]),
                                                uvo0, MUL)
                        nc.vector.tensor_tensor(_v3(dtu_1[:]), _v3(dt_1[:]),
                                                uvo1, MUL)

                    # ---- decays: a1 = exp(-dt); powers on DVE ----
                    a_d0 = [abp.tile([D0, LCH], BF16, name=f"a0_{n}",
                                     tag=f"a0_{n}") for n in range(N)]
                    nc.scalar.activation(a_d0[0][:], dt_0[:], AF.Exp,
                                         bias=0.0, scale=-1.0)
                    nc.scalar.activation(a_d0[1][:], dt_0[:], AF.Exp,
                                         bias=0.0, scale=-2.0)
                    nc.vector.tensor_tensor(a_d0[2][:], a_d0[1][:],
                                            a_d0[0][:], MUL)
                    nc.vector.tensor_tensor(a_d0[3][:], a_d0[1][:],
                                            a_d0[1][:], MUL)
                    a_d1 = [abp.tile([128, LCH], BF16, name=f"a1_{j}",
                                     tag=f"a1_{j}") for j in range(2)]
                    # pair0 = (E1|E1) -> (E1|E2); pair1 = pair0^2 = (E2|E4),
                    # then lower half *= pair0 lower -> (E3|E4)
                    nc.scalar.activation(a_d1[0][:], dt_1[:], AF.Exp,
                                         bias=0.0, scale=-1.0)
                    nc.vector.tensor_tensor(a_d1[0][64:128, :],
                                            a_d1[0][64:128, :],
                                            a_d1[0][64:128, :], MUL)
                    nc.vector.tensor_tensor(a_d1[1][:], a_d1[0][:],
                                            a_d1[0][:], MUL)
                    nc.vector.tensor_tensor(a_d1[1][0:64, :],
                                            a_d1[1][0:64, :],
                                            a_d1[0][0:64, :], MUL)

                    # ---- B/C broadcasts via DRAM bounce ----
                    # stage rows 0:4 = B_n, 4:8 = C_n (aligned src rows)
                    stg = bc_stage[k, c]
                    bsrc = bass.AP(tensor=B_sb.tensor, offset=B_sb[:].offset,
                                   ap=[[32 * LCH, 4]] + list(B_sb[:].ap)[1:])
                    nc.sync.dma_start(stg[0:4, :], bsrc)
                    csrc = bass.AP(tensor=C_sb.tensor, offset=C_sb[:].offset,
                                   ap=[[32 * LCH, 4]] + list(C_sb[:].ap)[1:])
                    nc.scalar.dma_start(stg[4:8, :], csrc)
                    qeng = [nc.sync, nc.scalar, nc.sync, nc.scalar]
                    B_bc0 = [bcp.tile([D0, LCH], BF16, name=f"Bb0_{n}",
                                      tag=f"bc0_{n}") for n in range(N)]
                    C_bc0 = [bcp.tile([D0, LCH], BF16, name=f"Cb0_{n}",
                                      tag=f"bc0_{n}") for n in range(N)]
                    for n in range(N):
                        brow = stg[n:n + 1, :]
                        rep = bass.AP(tensor=brow.tensor, offset=brow.offset,
                                      ap=[[0, D0]] + list(brow.ap)[1:])
                        qeng[n % 4].dma_start(B_bc0[n][:], rep)
                        crow = stg[4 + n:5 + n, :]
                        repc = bass.AP(tensor=crow.tensor, offset=crow.offset,
                                       ap=[[0, D0]] + list(crow.ap)[1:])
                        qeng[(n + 1) % 4].dma_start(C_bc0[n][:], repc)
                    # d1 pair j holds n = 2j (lower half) | n = 2j+1 (upper)
                    B_bc1 = [bcp.tile([128, LCH], BF16, name=f"Bb1_{j}",
                                      tag=f"bc1_{j}") for j in range(2)]
                    C_bc1 = [bcp.tile([128, LCH], BF16, name=f"Cb1_{j}",
                                      tag=f"bc1_{j}") for j in range(2)]
                    for j in range(2):
                        for half in range(2):
                            n = 2 * j + half
                            hs = slice(64 * half, 64 * half + 64)
                            brow = stg[n:n + 1, :]
                            rep = bass.AP(tensor=brow.tensor,
                                          offset=brow.offset,
                                          ap=[[0, 64]] + list(brow.ap)[1:])
                            qeng[(2 + j + half) % 4].dma_start(
                                B_bc1[j][hs, :], rep)
                            crow = stg[4 + n:5 + n, :]
                            repc = bass.AP(tensor=crow.tensor,
                                           offset=crow.offset,
                                           ap=[[0, 64]] + list(crow.ap)[1:])
                            qeng[(3 + j + half) % 4].dma_start(
                                C_bc1[j][hs, :], repc)

                    # ---- b inputs, scans, readout ----
                    scans = []
                    for n in range(N):
                        bt = abp.tile([D0, LCH], BF16, name=f"b0_{n}",
                                      tag=f"b0_{n}")
                        b_eng.tensor_tensor(bt[:], dtu_0[:], B_bc0[n][:], MUL)
                        ht = hp.tile([D0, LCH], BF16, name=f"h0_{n}",
                                     tag=f"h0_{n}")
                        scans.append((("d0", n), a_d0[n], bt, ht, C_bc0[n]))
                    for j in range(2):
                        bt = abp.tile([128, LCH], BF16, name=f"b1_{j}",
                                      tag=f"b1_{j}")
                        b_eng.tensor_tensor(bt[:], dtu_1[:], B_bc1[j][:], MUL)
                        ht = hp.tile([128, LCH], BF16, name=f"h1_{j}",
                                     tag=f"h1_{j}")
                        scans.append((("d1", j), a_d1[j], bt, ht, C_bc1[j]))

                    for key_sfx, at, bt, ht, _c in scans:
                        key = (k,) + key_sfx
                        init = carries.get(key, 0.0)
                        if not rev:
                            nc.vector.tensor_tensor_scan(
                                ht[:], at[:], bt[:], init, MUL, ADD)
                            carries[key] = ht[:, LCH - 1:LCH]
                        else:
                            nc.vector.tensor_tensor_scan(
                                ht[:, ::-1], at[:, ::-1], bt[:, ::-1],
                                init, MUL, ADD)
                            carries[key] = ht[:, 0:1]

                    hc0 = [rop.tile([D0, LCH], BF16, name=f"hc0_{n}",
                                    tag=f"hc0_{n}") for n in range(N)]
                    for i in range(N):
                        _key, at, bt, ht, cbt = scans[i]
                        hc_eng.tensor_tensor(hc0[i][:], ht[:], cbt[:], MUL)
                    s01 = rop.tile([D0, LCH], BF16, name="s01", tag="s01")
                    nc.vector.tensor_tensor(s01[:], hc0[0][:], hc0[1][:], ADD)
                    s23 = rop.tile([D0, LCH], BF16, name="s23", tag="s23")
                    nc.vector.tensor_tensor(s23[:], hc0[2][:], hc0[3][:], ADD)
                    s03 = rop.tile([D0, LCH], BF16, name="s03", tag="s03")
                    nc.vector.tensor_tensor(s03[:], s01[:], s23[:], ADD)
                    lc = c if k in (0, 1) else NCH - 1 - c
                    csl = slice(lc * LCH, (lc + 1) * LCH)
                    ydst0 = y02_d0 if k in (0, 2) else y13_d0
                    nc.vector.tensor_tensor(ydst0[:, csl], ydst0[:, csl],
                                            s03[:], ADD)

                    hc1 = [rop.tile([128, LCH], BF16, name=f"hc1_{j}",
                                    tag=f"hc1_{j}") for j in range(2)]
                    for j in range(2):
                        _key, at, bt, ht, cbt = scans[N + j]
                        hc_eng.tensor_tensor(hc1[j][:], ht[:], cbt[:], MUL)
                    sp = rop.tile([128, LCH], BF16, name="sp", tag="sp")
                    nc.vector.tensor_tensor(sp[:], hc1[0][:], hc1[1][:], ADD)
                    ydst1 = y02_d1p if k in (0, 2) else y13_d1p
                    nc.vector.tensor_tensor(ydst1[:, csl], ydst1[:, csl],
                                            sp[:], ADD)

        # ================= stage C: merge + LN + out_proj ==================
        with tc.tile_pool(name="merge", bufs=1) as mpool, \
             tc.tile_pool(name="merge2", bufs=1) as m2pool, \
             tc.tile_pool(name="merge_ps", bufs=2, space="PSUM") as mps:

            y_d0 = mpool.tile([D0, L], F32, name="y_d0")
            nc.vector.tensor_tensor(
                y_d0[:].rearrange("p (h w) -> p h w", w=W),
                y02_d0[:].rearrange("p (h w) -> p h w", w=W),
                y13_d0[:].rearrange("p (w h) -> p h w", w=W), ADD)
            nc.vector.affine_then_add(y_d0[:], u_o_d0[:], y_d0[:],
                                      w_dsum0[:], 0.0)
            y02_hi = mpool.tile([D1, L], BF16, name="y02_hi")
            nc.sync.dma_start(y02_hi[:], y02_d1p[64:128, :])
            y13_hi = mpool.tile([D1, L], BF16, name="y13_hi")
            nc.sync.dma_start(y13_hi[:], y13_d1p[64:128, :])
            y02_1 = mpool.tile([D1, L], BF16, name="y02_1")
            nc.vector.tensor_tensor(y02_1[:], y02_d1p[0:64, :],
                                    y02_hi[:], ADD)
            y13_1 = mpool.tile([D1, L], BF16, name="y13_1")
            nc.vector.tensor_tensor(y13_1[:], y13_d1p[0:64, :],
                                    y13_hi[:], ADD)
            y_d1 = mpool.tile([D1, L], F32, name="y_d1")
            nc.vector.tensor_tensor(
                y_d1[:].rearrange("p (h w) -> p h w", w=W),
                y02_1[:].rearrange("p (h w) -> p h w", w=W),
                y13_1[:].rearrange("p (w h) -> p h w", w=W), ADD)
            nc.vector.affine_then_add(y_d1[:], u_o_d1p[0:64, :], y_d1[:],
                                      w_dsum1[:], 0.0)

            y_bf0 = mpool.tile([D0, L], BF16, name="y_bf0")
            nc.vector.tensor_copy(y_bf0[:], y_d0[:])
            y_bf1 = mpool.tile([D1, L], BF16, name="y_bf1")
            nc.vector.tensor_copy(y_bf1[:], y_d1[:])
            y2_bf0 = mpool.tile([D0, L], BF16, name="y2_bf0")
            nc.scalar.activation(y2_bf0[:], y_d0[:], AF.Square)
            y2_bf1 = mpool.tile([D1, L], BF16, name="y2_bf1")
            nc.scalar.activation(y2_bf1[:], y_d1[:], AF.Square)

            lny0 = mpool.tile([D0, L], BF16, name="lny0")
            lny1 = mpool.tile([D1, L], BF16, name="lny1")
            for ci in range(L // MMCH):
                ms = slice(ci * MMCH, (ci + 1) * MMCH)
                mu_ps = mps.tile([128, MMCH], F32, name="mu_ps", tag="mu_ps")
                nc.tensor.matmul(mu_ps[:], mean_l0[:], y_bf0[:, ms],
                                 start=True, stop=False)
                nc.tensor.matmul(mu_ps[:], mean_l1[:], y_bf1[:, ms],
                                 start=False, stop=True)
                sq_ps = mps.tile([128, MMCH], F32, name="sq_ps", tag="sq_ps")
                nc.tensor.matmul(sq_ps[:], mean_l0[:], y2_bf0[:, ms],
                                 start=True, stop=False)
                nc.tensor.matmul(sq_ps[:], mean_l1[:], y2_bf1[:, ms],
                                 start=False, stop=True)
                mu_sb = m2pool.tile([128, MMCH], F32, name="mu_sb",
                                    tag="mu_sb")
                nc.vector.tensor_copy(mu_sb[:], mu_ps[:])
                var_t = m2pool.tile([128, MMCH], F32, name="var_t",
                                    tag="var_t")
                nc.vector.scalar_tensor_tensor(var_t[:], mu_sb[:], -1.0,
                                               mu_ps[:], MUL, MUL)
                nc.vector.tensor_tensor(var_t[:], sq_ps[:], var_t[:], ADD)
                lnv = m2pool.tile([128, MMCH], F32, name="lnv", tag="lnv")
                nc.scalar.activation(lnv[:], var_t[:], AF.Ln, bias=eps_col[:])
                rstd = m2pool.tile([128, MMCH], F32, name="rstd", tag="rstd")
                nc.scalar.activation(rstd[:], lnv[:], AF.Exp, bias=0.0,
                                     scale=-0.5)
                for part, ybf, lny, g, bb in (
                    (D0, y_bf0, lny0, w_lng0, w_lnb0),
                    (D1, y_bf1, lny1, w_lng1, w_lnb1),
                ):
                    ymu = m2pool.tile([128, MMCH], F32, name="ymu",
                                      tag=f"ymu{part}")
                    nc.vector.tensor_tensor(ymu[:part, :], ybf[:, ms],
                                            mu_sb[:part, :], SUB)
                    nc.vector.tensor_tensor(ymu[:part, :], ymu[:part, :],
                                            rstd[:part, :], MUL)
                    nc.vector.tensor_scalar(lny[:, ms], ymu[:part, :],
                                            g[:], bb[:], MUL, ADD)

            with tc.tile_pool(name="outp", bufs=3) as opool, \
                 tc.tile_pool(name="outp_ps", bufs=2, space="PSUM") as ops:
                for ci in range(L // 128):
                    ls = slice(ci * 128, (ci + 1) * 128)
                    pso = ops.tile([128, C], F32, name="pso", tag="pso")
                    nc.tensor.matmul(pso[:], lny0[:, ls], w_woutT0[:],
                                     start=True, stop=False)
                    nc.tensor.matmul(pso[:], lny1[:, ls], w_woutT1[:],
                                     start=False, stop=True)
                    res = opool.tile([128, C], F32, name="res", tag="res")
                    nc.sync.dma_start(res[:], xnat_o[ls, :])
                    outt = opool.tile([128, C], F32, name="outt", tag="outt")
                    nc.vector.tensor_tensor(outt[:], pso[:], res[:], ADD)
                    nc.sync.dma_start(out_o[ls, :], outt[:])

    nc.finalize()
    return nc


_CACHE = {}


def _kperm(a):
    """[K, P, M] -> [P, K*M] bf16 (k-major along free)."""
    return np.ascontiguousarray(
        np.transpose(a, (1, 0, 2)).reshape(a.shape[1], -1)).astype(BF)


def _prep_core_inputs(inputs, b, mod):
    x_own = inputs["x_rgb"] if mod == 0 else inputs["x_e"]
    x_oth = inputs["x_e"] if mod == 0 else inputs["x_rgb"]
    ipw_own = inputs["in_proj_x_w"] if mod == 0 else inputs["in_proj_e_w"]
    ipw_oth = inputs["in_proj_e_w"] if mod == 0 else inputs["in_proj_x_w"]
    cw_own = inputs["conv_x_w"] if mod == 0 else inputs["conv_e_w"]
    cw_oth = inputs["conv_e_w"] if mod == 0 else inputs["conv_x_w"]
    cb_own = inputs["conv_x_b"] if mod == 0 else inputs["conv_e_b"]
    cb_oth = inputs["conv_e_b"] if mod == 0 else inputs["conv_x_b"]
    lng = inputs["ln_r_g"] if mod == 0 else inputs["ln_e_g"]
    lnb = inputs["ln_r_b"] if mod == 0 else inputs["ln_e_b"]
    wout = inputs["out_proj_x_w"] if mod == 0 else inputs["out_proj_e_w"]

    def padT(x):
        xp = np.zeros((C, H + 2, W + 2), np.float32)
        xp[:, 1:H + 1, 1:W + 1] = np.transpose(x, (2, 0, 1))
        return xp.reshape(C, -1).astype(BF)

    def fused_w(ipw, cw):
        # [C, 9*256]; per tap: cols 0:128 = d0; 128:192 = d1; 192:256 = d1 dup
        wf = np.zeros((9, C, 256), np.float32)
        for tap in range(9):
            dy, dx = tap // 3, tap % 3
            full = ipw.T * cw[:, 0, dy, dx][None, :]      # [C, DIN]
            wf[tap, :, :128] = full[:, :128]
            wf[tap, :, 128:192] = full[:, 128:]
            wf[tap, :, 192:256] = full[:, 128:]
        return np.ascontiguousarray(
            np.transpose(wf, (1, 0, 2)).reshape(C, 9 * 256)).astype(BF)

    def dup256(v):
        out = np.zeros(256, np.float32)
        out[:128] = v[:128]
        out[128:192] = v[128:]
        out[192:256] = v[128:]
        return out

    xpw = inputs["x_proj_weight"]
    dtw = inputs["dt_projs_weight"]
    dtb = inputs["dt_projs_bias"]
    Ds = inputs["Ds"]

    xpw_dt = np.transpose(xpw[:, :R, :], (0, 2, 1))      # [K, DIN, R]
    xpw_Bp = np.zeros((K, DIN, 128), np.float32)
    xpw_Cp = np.zeros((K, DIN, 128), np.float32)
    for n in range(N):
        xpw_Bp[:, :, 32 * n] = xpw[:, R + n, :]
        xpw_Cp[:, :, 32 * n] = xpw[:, R + N + n, :]
    dtw_t = np.transpose(dtw, (0, 2, 1))                 # [K, R, DIN]
    dtw_d1p = np.concatenate([dtw_t[:, :, 128:], dtw_t[:, :, 128:]], axis=2)
    dtb_d1p = np.concatenate([dtb[:, 128:], dtb[:, 128:]], axis=1)  # [K, 128]
    dsum = Ds.reshape(K, DIN).sum(axis=0)

    f32 = np.float32
    return {
        "xpad_o": padT(x_own[b]),
        "xpad_t": padT(x_oth[b]),
        "xnat_o": np.ascontiguousarray(x_own[b].reshape(L, C)).astype(f32),
        "wf_o": fused_w(ipw_own, cw_own),
        "wf_t": fused_w(ipw_oth, cw_oth),
        "cb_o": dup256(cb_own)[None, :].astype(BF),
        "cb_t": dup256(cb_oth)[None, :].astype(BF),
        "xpw_dt0": _kperm(xpw_dt[:, :128, :]),
        "xpw_dt1": _kperm(xpw_dt[:, 128:, :]),
        "xpw_B0": _kperm(xpw_Bp[:, :128, :]),
        "xpw_B1": _kperm(xpw_Bp[:, 128:, :]),
        "xpw_C0": _kperm(xpw_Cp[:, :128, :]),
        "xpw_C1": _kperm(xpw_Cp[:, 128:, :]),
        "dtw_d0": _kperm(dtw_t[:, :, :128]),
        "dtw_d1p": _kperm(dtw_d1p),
        "dtb_d0": np.ascontiguousarray(dtb[:, :128].T).astype(f32),
        "dtb_d1p": np.ascontiguousarray(dtb_d1p.T).astype(f32),
        "dsum_d0": dsum[:128, None].astype(f32),
        "dsum_d1": dsum[128:, None].astype(f32),
        "ln_g0": lng[:128, None].astype(f32),
        "ln_g1": lng[128:, None].astype(f32),
        "ln_b0": lnb[:128, None].astype(f32),
        "ln_b1": lnb[128:, None].astype(f32),
        "woutT0": np.ascontiguousarray(wout.T[:128, :]).astype(BF),
        "woutT1": np.ascontiguousarray(wout.T[128:, :]).astype(BF),
    }


def kernel(**inputs):
    if "nc" not in _CACHE:
        _CACHE["nc"] = build_nc()
    nc = _CACHE["nc"]
    in_maps = [_prep_core_inputs(inputs, core // 2, core % 2)
               for core in range(NCORE)]
    res = run_bass_kernel_spmd(nc, in_maps, core_ids=list(range(NCORE)))
    _CACHE["last_res"] = res
    out = np.empty((2, B, H, W, C), np.float32)
    for core in range(NCORE):
        b, mod = core // 2, core % 2
        out[mod, b] = res.results[core]["out_o"].reshape(H, W, C)
    return out


if __name__ == "__main__":
    build_nc()
    print("build ok")
